# revision 21
# baseline (speedup 1.0000x reference)
"""Trainium2 Bass kernel for nn_CNNMnist_Sketch (sketched CNN forward pass), v3.

Data-parallel over 8 NeuronCores: batch 4096 -> 512 per core.
Per-core pipeline (all shapes hardcoded):
  conv1 5x5 (1->32ch) + maxpool2 + relu   -> h1  [32ch, 12x12]
  conv2 5x5 (32->64ch) + maxpool2 + relu  -> h2  [64ch, 4x4] -> flat 1024
  fc1 1024->512 + relu, fc2 512->10, log_softmax

v3 changes vs v2 (362 us -> 321 us):
  - h1 DRAM bounce eliminated: conv1 output partition order is 32j+ci so
    the 4 shifted conv2 input copies are direct SBUF->SBUF DMAs (one per
    (copy, chunk), 32 partitions each -- DMA partition dims must be a
    clean outermost run).  Cuts ~23.6 MB/core of HBM round-trip.
  - conv2 taps packed 2x2: copies hold shifts {0,1,12,13} (post-pool
    row/col), passes sweep offsets {0,2,4}x{0,2,4} -> 9 passes of K=128
    instead of 6xK=128 + 6xK=32 (PE -25%).
  - conv1 PSUM in 3-bank groups [128,1536]; bias+relu+cast eviction is
    one batched ACT op per group; 2x2 pool is one DVE tensor_reduce per
    group-pair (1728 elems) to amortize the ~300c fixed overheads.
    (gpsimd ALU ops don't lower in this walrus build.)
  - all bulk DMA on the HWDGE (sync) ring; gpsimd does no DMA issue.
  - conv2 col-pool (stt2) APs enumerated (rp,cp,s) so the h2 write's
    inner dim is a contiguous 16-sample run.
  - fc1 split into batch halves so the first half overlaps the tail
    conv2 blocks; fc1/fc2 otherwise as v2.
"""

import numpy as np
import ml_dtypes

import concourse.bass as bass
import concourse.bacc as bacc
import concourse.tile as tile
from concourse import mybir
from concourse.bass_utils import run_bass_kernel_spmd

F32 = mybir.dt.float32
BF16 = mybir.dt.bfloat16
RELU = mybir.ActivationFunctionType.Relu
EXP = mybir.ActivationFunctionType.Exp
LN = mybir.ActivationFunctionType.Ln
MAXOP = mybir.AluOpType.max
SUBOP = mybir.AluOpType.subtract
ADDOP = mybir.AluOpType.add
AXY = mybir.AxisListType.XY
AX = mybir.AxisListType.X

NCORES = 8
BPC = 4096 // NCORES          # samples per core
BLK = 64                      # samples per block
NBLK = BPC // BLK
CS = BLK // 4                 # samples per conv1 chunk (4 chunks / block)
CHUNKF = CS * 784             # x elements per chunk
XBLK = BLK * 784              # x elements per block
XP = CHUNKF + 8               # xrep free pitch
H1F = CS * 144                # h1 elements per chunk (per channel)
P1 = H1F + 16                 # h1p free pitch (pad >= 13 for copy shifts)
PR = 4 * H1F + 16             # h1r free pitch
XPAD = 128                    # DRAM pad so shifted reads never go OOB

# conv2 pass offsets (a=row*12, b=col), 2x2 tap packing
PASSES = [(a, b) for b in (0, 2, 4) for a in (0, 2, 4)]
# copy c = 2*alpha+beta holds h1 shifted by 12*alpha + beta
COPIES = [(0, 0), (0, 1), (1, 0), (1, 1)]

# conv1 group sizes (32 matmuls per block in 11 groups; psum = 3 banks)
GSIZES = [3] * 10 + [2]
# eviction engine per group: 'A' = scalar(ACT), 'D' = vector(DVE)
# (gpsimd ALU ops don't lower in this walrus build; pool is DVE-only,
#  batched over evict-group pairs to amortize the ~300c reduce overhead)
ENG_EVICT = ['A'] * 11

_CACHE = {}
DEBUG = False


def _build():
    nc = bacc.Bacc(target_bir_lowering=False, debug=False, num_devices=NCORES)

    xt = nc.dram_tensor("x", [BPC * 784 + XPAD], BF16, kind="ExternalInput").ap()
    wc1t = nc.dram_tensor("wc1bd", [100, 128], BF16, kind="ExternalInput").ap()
    w2t = nc.dram_tensor("w2s", [128, 1152], BF16, kind="ExternalInput").ap()
    w3t = nc.dram_tensor("w3sb", [128, 4096], BF16, kind="ExternalInput").ap()
    fc2t = nc.dram_tensor("fc2sb", [128, 40], F32, kind="ExternalInput").ap()
    b1t = nc.dram_tensor("b1r", [128, 1], F32, kind="ExternalInput").ap()
    b2t = nc.dram_tensor("b2", [64, 1], F32, kind="ExternalInput").ap()
    b3t = nc.dram_tensor("b3sb", [128, 4], F32, kind="ExternalInput").ap()
    fbt = nc.dram_tensor("fc2b", [1, 10], F32, kind="ExternalInput").ap()
    ot = nc.dram_tensor("out", [BPC, 10], F32, kind="ExternalOutput").ap()
    dbg1 = dbg2 = None
    if DEBUG:
        dbg1 = nc.dram_tensor("dbg1", [128, P1], F32, kind="ExternalOutput").ap()
        dbg2 = nc.dram_tensor("dbg2", [128, PR], F32, kind="ExternalOutput").ap()
        dbg3 = nc.dram_tensor("dbg3", [64, 16 * BPC], F32, kind="ExternalOutput").ap()
        dbg4 = nc.dram_tensor("dbg4", [128, 512], F32, kind="ExternalOutput").ap()
        dbg5 = nc.dram_tensor("dbg5", [128, 512], F32, kind="ExternalOutput").ap()

    from contextlib import ExitStack

    with tile.TileContext(nc, num_cores=NCORES) as tc, ExitStack() as es:
        W = es.enter_context(tc.tile_pool(name="weights", bufs=1))
        S = es.enter_context(tc.tile_pool(name="work", bufs=2))
        P = es.enter_context(tc.tile_pool(name="persist", bufs=1))
        PS = es.enter_context(tc.tile_pool(name="ps", bufs=1, space="PSUM"))

        # ---- load weights ----
        wc1 = W.tile([100, 128], BF16)
        nc.sync.dma_start(out=wc1[:], in_=wc1t)
        w2s = W.tile([128, 1152], BF16)
        nc.sync.dma_start(out=w2s[:], in_=w2t)
        w3 = W.tile([128, 4096], BF16)
        nc.sync.dma_start(out=w3[:], in_=w3t)
        fc2 = W.tile([128, 40], F32)
        nc.sync.dma_start(out=fc2[:], in_=fc2t)
        b1r = W.tile([128, 1], F32)
        nc.sync.dma_start(out=b1r[:], in_=b1t)
        b2 = W.tile([64, 1], F32)
        nc.sync.dma_start(out=b2[:], in_=b2t)
        b3 = W.tile([128, 4], F32)
        nc.sync.dma_start(out=b3[:], in_=b3t)
        fc2b = W.tile([1, 10], F32)
        nc.sync.dma_start(out=fc2b[:], in_=fbt)
        ones1 = W.tile([1, 128], F32)
        nc.vector.memset(ones1[:], 1.0)
        # touch ln/exp once now so their ACT table loads (~2.7us each)
        # overlap the conv phase instead of the serial fc2 tail
        warm = W.tile([1, 2], F32)
        nc.scalar.activation(warm[:], ones1[:, 0:2], LN)
        nc.scalar.activation(warm[:], ones1[:, 0:2], EXP)

        h2 = P.tile([64, 16 * BPC], BF16)          # free = (sp outer, b inner)

        def load_x(blk):
            # conv1 input: 25 shifted replicas per chunk straight from DRAM
            # partition 25j+5kh+kw = chunk j shifted by 28*kh + kw.
            xrep = S.tile([100, XP], BF16, tag="xrep", bufs=3, name="xrep")
            for j in range(4):
                srcR = bass.AP(
                    tensor=xt.tensor,
                    offset=blk * XBLK + j * CHUNKF,
                    ap=[[28, 5], [1, 5], [1, CHUNKF]],
                )
                nc.sync.dma_start(
                    out=xrep[25 * j : 25 * j + 25, 0:CHUNKF], in_=srcR
                )
            return xrep

        def conv1_block(blk, xrep, prev):
            # psum partition m = 32*j + ci (j = chunk, ci = conv1 out ch)
            h1p = S.tile([128, P1], BF16, tag="h1p", name="h1p")
            # zero the pad tail once per block: the shifted copies read
            # up to 13 elements past H1F and NaN*0 = NaN in the PE.
            nc.vector.memset(h1p[:, H1F:P1], 0.0)
            m0 = 0
            c1t = None
            pstart = 0
            for g, gs in enumerate(GSIZES):
                ps1 = PS.tile([128, 1536], F32, tag="ps1", bufs=2, name="ps1")
                for i in range(gs):
                    m = m0 + i
                    s, h = m // 2, m % 2
                    rhs = bass.AP(
                        tensor=xrep[:].tensor,
                        offset=xrep[:].offset + s * 784 + h * 336,
                        ap=[[XP, 100], [28, 12], [1, 24]],
                    )
                    nc.tensor.matmul(
                        out=ps1[:, 512 * i : 512 * i + 288],
                        lhsT=wc1[:],
                        rhs=rhs,
                        start=True,
                        stop=True,
                    )
                # batched bias+relu+cast eviction over the group's banks;
                # c1t holds two groups (6 matmuls) so the pool reduce
                # amortizes its fixed overhead over 1728 elements.
                if c1t is None:
                    c1t = S.tile([128, 1728], BF16, tag="c1t", bufs=2,
                                 name="c1t")
                    pstart = m0
                cbase = (m0 - pstart) * 288
                pin = bass.AP(
                    tensor=ps1[:].tensor,
                    offset=ps1[:].offset,
                    ap=[[1536, 128], [512, gs], [1, 288]],
                )
                cout = bass.AP(
                    tensor=c1t[:].tensor,
                    offset=c1t[:].offset + cbase,
                    ap=[[1728, 128], [288, gs], [1, 288]],
                )
                if ENG_EVICT[g] == 'A':
                    nc.scalar.activation(cout, pin, RELU, bias=b1r[:])
                else:
                    nc.vector.tensor_scalar(
                        out=cout, in0=pin, scalar1=b1r[:], scalar2=0.0,
                        op0=ADDOP, op1=MAXOP,
                    )
                # 2x2 max pool once per c1t fill (6 mms, or the 2mm tail):
                # c1t free = (m_local, ph, s1, pw, s0)
                #   f = ml*288 + ph*48 + s1*24 + pw*2 + s0
                # (ml, ph) merge: stride 48, size 6*nm.
                nm = m0 + gs - pstart
                if nm == 6 or g == len(GSIZES) - 1:
                    ov = bass.AP(
                        tensor=h1p[:].tensor,
                        offset=h1p[:].offset + pstart * 72,
                        ap=[[P1, 128], [12, 6 * nm], [1, 12]],
                    )
                    pv = bass.AP(
                        tensor=c1t[:].tensor,
                        offset=c1t[:].offset,
                        ap=[[1728, 128], [48, 6 * nm], [2, 12], [24, 2], [1, 2]],
                    )
                    nc.vector.tensor_reduce(out=ov, in_=pv, axis=AXY, op=MAXOP)
                    c1t = None
                # interleave previous block's conv2 chunks into the PE stream
                if prev is not None and g in (1, 3, 5, 7):
                    conv2_j(prev[0], prev[1], (1, 3, 5, 7).index(g))
                m0 += gs

            # 4 shifted SBUF->SBUF copies: h1r partition 32c+ci holds
            # chunk j at free j*H1F, shifted by 12*alpha + beta.
            h1r = S.tile([128, PR], BF16, tag="h1r", bufs=3, name="h1r")
            for c, (al, be) in enumerate(COPIES):
                sc = 12 * al + be
                for j in range(4):
                    # partition dims must be a clean outermost run: one
                    # DMA per (copy, chunk), 32 partitions each.
                    src = bass.AP(
                        tensor=h1p[:].tensor,
                        offset=h1p[:].offset + 32 * j * P1 + sc,
                        ap=[[P1, 32], [1, H1F]],
                    )
                    dst = bass.AP(
                        tensor=h1r[:].tensor,
                        offset=h1r[:].offset + 32 * c * PR + j * H1F,
                        ap=[[PR, 32], [1, H1F]],
                    )
                    nc.sync.dma_start(out=dst, in_=src)
            if DEBUG and blk == 0:
                nc.gpsimd.dma_start(out=dbg1, in_=h1p[:])
                nc.gpsimd.dma_start(out=dbg2, in_=h1r[:])
            return h1r

        def conv2_j(blk, h1r, j):
            # 9 delta-packed passes, K=128 each, one N=512 matmul per pass
            ps2 = PS.tile([128, 512], F32, tag="ps2", bufs=2, name="ps2")
            for p, (a, b) in enumerate(PASSES):
                rhs = bass.AP(
                    tensor=h1r[:].tensor,
                    offset=h1r[:].offset + j * H1F + 12 * a + b,
                    ap=[[PR, 128], [144, 16], [24, 4], [1, 8]],
                )
                nc.tensor.matmul(
                    out=ps2[:],
                    lhsT=w2s[:, 128 * p : 128 * p + 128],
                    rhs=rhs,
                    start=(p == 0),
                    stop=(p == 8),
                )
            if DEBUG and blk == 0 and j in (1, 3):
                d4t = S.tile([128, 512], F32, tag="d4t", name="d4t")
                nc.scalar.activation(
                    d4t[:], ps2[:],
                    mybir.ActivationFunctionType.Identity,
                )
                nc.gpsimd.dma_start(out=(dbg4 if j == 1 else dbg5), in_=d4t[:])
            # pool rows: max over delta halves (partitions p vs p+64),
            # bias folded in; one PSUM input per instruction
            t2 = S.tile([64, 512], F32, tag="t2", name="t2")
            nc.scalar.activation(
                t2[:], ps2[64:128, :],
                mybir.ActivationFunctionType.Identity, bias=b2[:],
            )
            m1 = S.tile([64, 512], BF16, tag="m1", name="m1")
            nc.vector.scalar_tensor_tensor(
                out=m1[:], in0=ps2[0:64, :], scalar=b2[:], in1=t2[:],
                op0=ADDOP, op1=MAXOP,
            )
            # pool cols: max(m1[2oc'], 0, m1[2oc'+1]) -> relu folded,
            # written straight into h2[co, sp*BPC + b]
            b0 = blk * BLK + j * CS
            ia = bass.AP(
                tensor=m1[:].tensor,
                offset=m1[:].offset,
                ap=[[512, 64], [8, 4], [2, 4], [32, 16]],
            )
            ib = bass.AP(
                tensor=m1[:].tensor,
                offset=m1[:].offset + 1,
                ap=[[512, 64], [8, 4], [2, 4], [32, 16]],
            )
            outv = bass.AP(
                tensor=h2[:].tensor,
                offset=h2[:].offset + b0,
                ap=[[16 * BPC, 64], [4 * BPC, 4], [BPC, 4], [1, 16]],
            )
            nc.vector.scalar_tensor_tensor(
                out=outv, in0=ia, scalar=0.0, in1=ib, op0=MAXOP, op1=MAXOP
            )

        hr = [P.tile([128, BPC + 8], BF16, tag=f"hr{k}", name=f"hr{k}")
              for k in range(8)]

        def relayout_half(bh):
            for k in range(8):
                src = bass.AP(
                    tensor=h2[:].tensor,
                    offset=h2[:].offset + 8 * k * 16 * BPC + bh * (BPC // 2),
                    ap=[[16 * BPC, 8], [BPC, 16], [1, BPC // 2]],
                )
                nc.sync.dma_start(
                    out=hr[k][:, bh * (BPC // 2) : (bh + 1) * (BPC // 2)],
                    in_=src,
                )

        # ---- main pipeline ----
        xreps = [load_x(0), load_x(1)]
        h1rs = []
        for blk in range(NBLK):
            if blk + 2 < NBLK:
                xreps.append(load_x(blk + 2))
            prev = (blk - 2, h1rs[blk - 2]) if blk >= 2 else None
            h1rs.append(conv1_block(blk, xreps[blk], prev))
        for j in range(4):
            conv2_j(NBLK - 2, h1rs[NBLK - 2], j)
        relayout_half(0)
        for j in range(4):
            conv2_j(NBLK - 1, h1rs[NBLK - 1], j)
        relayout_half(1)
        if DEBUG:
            nc.gpsimd.dma_start(out=dbg3, in_=h2[:])

        # ---- fc1: 8 K-chunks, batch halves so half 0 overlaps tail ----
        h3 = [P.tile([128, BPC], F32, tag=f"h3{m}", name=f"h3{m}")
              for m in range(4)]
        HB = BPC // 2
        for bh in range(2):
            for mh in range(2):
                psf = [PS.tile([128, 512], F32, tag="ps2", bufs=2,
                               name=f"psf{bh}{mh}{m}") for m in range(2)]
                for k in range(8):
                    for mm in range(2):
                        m = 2 * mh + mm
                        nc.tensor.matmul(
                            out=psf[mm][:, 0:HB],
                            lhsT=w3[:, (k * 4 + m) * 128 : (k * 4 + m) * 128 + 128],
                            rhs=hr[k][:, bh * HB : (bh + 1) * HB],
                            start=(k == 0),
                            stop=(k == 7),
                        )
                for mm in range(2):
                    m = 2 * mh + mm
                    nc.scalar.activation(
                        h3[m][:, bh * HB : (bh + 1) * HB],
                        psf[mm][:, 0:HB], RELU, bias=b3[:, m : m + 1],
                    )

        # ---- fc2 + log_softmax, batch on partitions ----
        for bc in range(4):
            psl = PS.tile([128, 10], F32, tag="ps1", bufs=2)
            for k in range(4):
                nc.tensor.matmul(
                    out=psl[:],
                    lhsT=h3[k][:, bc * 128 : bc * 128 + 128],
                    rhs=fc2[:, k * 10 : k * 10 + 10],
                    start=(k == 0),
                    stop=False,
                )
            nc.tensor.matmul(
                out=psl[:],
                lhsT=ones1[:],
                rhs=fc2b[:],
                start=False,
                stop=True,
            )
            negm = S.tile([128, 1], F32, tag="negm")
            nc.vector.tensor_reduce(
                out=negm[:], in_=psl[:], axis=AX, op=MAXOP, negate=True
            )
            shifted = S.tile([128, 10], F32, tag="shifted")
            nc.vector.tensor_scalar(
                out=shifted[:], in0=psl[:], scalar1=negm[:], scalar2=None, op0=ADDOP
            )
            ex = S.tile([128, 10], F32, tag="ex")
            se = S.tile([128, 1], F32, tag="se")
            nc.scalar.activation(ex[:], shifted[:], EXP, accum_out=se[:])
            lse = S.tile([128, 1], F32, tag="lse")
            nc.scalar.activation(lse[:], se[:], LN)
            osb = S.tile([128, 10], F32, tag="osb")
            nc.vector.tensor_scalar(
                out=osb[:], in0=shifted[:], scalar1=lse[:], scalar2=None, op0=SUBOP
            )
            nc.sync.dma_start(out=ot[bc * 128 : bc * 128 + 128, :], in_=osb[:])

    nc.finalize()
    return nc


def _prep_weights(inputs):
    """Host-side: densify sketch weights and lay them out for the kernel."""
    h1, h2i, h3i = inputs["hash_idx1"], inputs["hash_idx2"], inputs["hash_idx3"]
    s1, s2, s3 = inputs["sgn1"], inputs["sgn2"], inputs["sgn3"]
    w1, w2, w3 = inputs["w1"], inputs["w2"], inputs["w3"]
    b1, b2, b3 = inputs["b1"], inputs["b2"], inputs["b3"]
    fc2w, fc2b = inputs["fc2_w"], inputs["fc2_b"]

    wc1 = (w1[:, h1] * s1[None, :]).astype(np.float32)            # (32, 25)
    wc2 = (w2[:, h2i] * s2[None, :]).astype(np.float32).reshape(64, 32, 5, 5)
    W3 = (w3[:, h3i] * s3[None, :]).astype(np.float32)            # (512, 1024)

    # conv1 block-diagonal; psum partition m = 32*j + ci
    wc1bd = np.zeros((100, 128), np.float32)
    for j in range(4):
        for ci in range(32):
            wc1bd[25 * j : 25 * j + 25, 32 * j + ci] = wc1[ci]
    b1r = np.tile(np.asarray(b1, np.float32), 4).reshape(128, 1)

    # conv2 2x2-packed weights: 9 passes (a,b); K row = 32*(2*al+be)+ci,
    # M col = 64*delta+co; tap kh = a+al-delta, kw = b+be.
    w2s = np.zeros((128, 9, 128), np.float32)
    for p, (a, b) in enumerate(PASSES):
        for c, (al, be) in enumerate(COPIES):
            for d in range(2):
                kh = a + al - d
                kw = b + be
                if 0 <= kh <= 4 and 0 <= kw <= 4:
                    w2s[32 * c : 32 * c + 32, p, 64 * d : 64 * d + 64] = \
                        wc2[:, :, kh, kw].T
    w2s = w2s.reshape(128, 1152)

    # fc1: lhsT chunk (k,m) = W3.T[128k:128k+128, 128m:128m+128]
    w3sb = np.zeros((128, 8, 4, 128), np.float32)
    W3T = np.ascontiguousarray(W3.T)  # (1024, 512)
    for k in range(8):
        for m in range(4):
            w3sb[:, k, m, :] = W3T[128 * k : 128 * k + 128, 128 * m : 128 * m + 128]
    w3sb = w3sb.reshape(128, 4096)

    fc2sb = np.zeros((128, 4, 10), np.float32)
    for k in range(4):
        fc2sb[:, k, :] = fc2w[:, 128 * k : 128 * k + 128].T
    fc2sb = fc2sb.reshape(128, 40)

    b3sb = np.asarray(b3, np.float32).reshape(4, 128).T.copy()

    bf = lambda a: np.asarray(a, dtype=ml_dtypes.bfloat16)
    f = lambda a: np.ascontiguousarray(a, dtype=np.float32)
    return {
        "wc1bd": bf(wc1bd),
        "w2s": bf(w2s),
        "w3sb": bf(w3sb),
        "fc2sb": f(fc2sb),
        "b1r": f(b1r),
        "b2": f(np.asarray(b2).reshape(64, 1)),
        "b3sb": f(b3sb),
        "fc2b": f(np.asarray(fc2b).reshape(1, 10)),
    }


def kernel(**inputs):
    out, _ = _run(inputs, trace=False)
    return out


def _run(inputs, trace=False):
    if "nc" not in _CACHE:
        _CACHE["nc"] = _build()
    nc = _CACHE["nc"]

    wmap = _prep_weights(inputs)
    x = np.asarray(inputs["x"], np.float32).reshape(4096, 784)

    in_maps = []
    for c in range(NCORES):
        xs = x[c * BPC : (c + 1) * BPC].reshape(-1)
        xs = np.concatenate([xs, np.zeros(XPAD, np.float32)])
        m = dict(wmap)
        m["x"] = np.asarray(xs, dtype=ml_dtypes.bfloat16)
        in_maps.append(m)

    res = run_bass_kernel_spmd(
        nc, in_maps, core_ids=list(range(NCORES)), trace=trace
    )
    out = np.concatenate([res.results[c]["out"] for c in range(NCORES)], axis=0)
    return out.astype(np.float32), res


# revision 22
# speedup vs baseline: 1.0121x; 1.0121x over previous
"""Trainium2 Bass kernel for nn_CNNMnist_Sketch (sketched CNN forward pass), v3.

Data-parallel over 8 NeuronCores: batch 4096 -> 512 per core.
Per-core pipeline (all shapes hardcoded):
  conv1 5x5 (1->32ch) + maxpool2 + relu   -> h1  [32ch, 12x12]
  conv2 5x5 (32->64ch) + maxpool2 + relu  -> h2  [64ch, 4x4] -> flat 1024
  fc1 1024->512 + relu, fc2 512->10, log_softmax

v3 changes vs v2 (362 us -> 321 us):
  - h1 DRAM bounce eliminated: conv1 output partition order is 32j+ci so
    the 4 shifted conv2 input copies are direct SBUF->SBUF DMAs (one per
    (copy, chunk), 32 partitions each -- DMA partition dims must be a
    clean outermost run).  Cuts ~23.6 MB/core of HBM round-trip.
  - conv2 taps packed 2x2: copies hold shifts {0,1,12,13} (post-pool
    row/col), passes sweep offsets {0,2,4}x{0,2,4} -> 9 passes of K=128
    instead of 6xK=128 + 6xK=32 (PE -25%).
  - conv1 PSUM in 3-bank groups [128,1536]; bias+relu+cast eviction is
    one batched ACT op per group; 2x2 pool is one DVE tensor_reduce per
    group-pair (1728 elems) to amortize the ~300c fixed overheads.
    (gpsimd ALU ops don't lower in this walrus build.)
  - all bulk DMA on the HWDGE (sync) ring; gpsimd does no DMA issue.
  - conv2 col-pool (stt2) APs enumerated (rp,cp,s) so the h2 write's
    inner dim is a contiguous 16-sample run.
  - fc1 split into batch halves so the first half overlaps the tail
    conv2 blocks; fc1/fc2 otherwise as v2.
"""

import numpy as np
import ml_dtypes

import concourse.bass as bass
import concourse.bacc as bacc
import concourse.tile as tile
from concourse import mybir
from concourse.bass_utils import run_bass_kernel_spmd

F32 = mybir.dt.float32
BF16 = mybir.dt.bfloat16
RELU = mybir.ActivationFunctionType.Relu
EXP = mybir.ActivationFunctionType.Exp
LN = mybir.ActivationFunctionType.Ln
MAXOP = mybir.AluOpType.max
SUBOP = mybir.AluOpType.subtract
ADDOP = mybir.AluOpType.add
AXY = mybir.AxisListType.XY
AX = mybir.AxisListType.X

NCORES = 8
BPC = 4096 // NCORES          # samples per core
BLK = 64                      # samples per block
NBLK = BPC // BLK
CS = BLK // 4                 # samples per conv1 chunk (4 chunks / block)
CHUNKF = CS * 784             # x elements per chunk
XBLK = BLK * 784              # x elements per block
XP = CHUNKF + 8               # xrep free pitch
H1F = CS * 144                # h1 elements per chunk (per channel)
P1 = H1F + 16                 # h1p free pitch (pad >= 13 for copy shifts)
PR = 4 * H1F + 16             # h1r free pitch
XPAD = 128                    # DRAM pad so shifted reads never go OOB

# conv2 pass offsets (a=row*12, b=col), 2x2 tap packing
PASSES = [(a, b) for b in (0, 2, 4) for a in (0, 2, 4)]
# copy c = 2*alpha+beta holds h1 shifted by 12*alpha + beta
COPIES = [(0, 0), (0, 1), (1, 0), (1, 1)]

# conv1 group sizes (32 matmuls per block in 11 groups; psum = 3 banks)
GSIZES = [3] * 10 + [2]
# eviction engine per group: 'A' = scalar(ACT), 'D' = vector(DVE)
# (gpsimd ALU ops don't lower in this walrus build; pool is DVE-only,
#  batched over evict-group pairs to amortize the ~300c reduce overhead)
ENG_EVICT = ['A'] * 11

_CACHE = {}
DEBUG = False


def _build():
    nc = bacc.Bacc(target_bir_lowering=False, debug=False, num_devices=NCORES)

    xt = nc.dram_tensor("x", [BPC * 784 + XPAD], BF16, kind="ExternalInput").ap()
    wc1t = nc.dram_tensor("wc1bd", [100, 128], BF16, kind="ExternalInput").ap()
    w2t = nc.dram_tensor("w2s", [128, 1152], BF16, kind="ExternalInput").ap()
    w3t = nc.dram_tensor("w3sb", [128, 4096], BF16, kind="ExternalInput").ap()
    fc2t = nc.dram_tensor("fc2sb", [128, 40], F32, kind="ExternalInput").ap()
    b1t = nc.dram_tensor("b1r", [128, 1], F32, kind="ExternalInput").ap()
    b2t = nc.dram_tensor("b2", [64, 1], F32, kind="ExternalInput").ap()
    b3t = nc.dram_tensor("b3sb", [128, 4], F32, kind="ExternalInput").ap()
    fbt = nc.dram_tensor("fc2b", [1, 10], F32, kind="ExternalInput").ap()
    ot = nc.dram_tensor("out", [BPC, 10], F32, kind="ExternalOutput").ap()
    dbg1 = dbg2 = None
    if DEBUG:
        dbg1 = nc.dram_tensor("dbg1", [128, P1], F32, kind="ExternalOutput").ap()
        dbg2 = nc.dram_tensor("dbg2", [128, PR], F32, kind="ExternalOutput").ap()
        dbg3 = nc.dram_tensor("dbg3", [64, 16 * BPC], F32, kind="ExternalOutput").ap()
        dbg4 = nc.dram_tensor("dbg4", [128, 512], F32, kind="ExternalOutput").ap()
        dbg5 = nc.dram_tensor("dbg5", [128, 512], F32, kind="ExternalOutput").ap()

    from contextlib import ExitStack

    with tile.TileContext(nc, num_cores=NCORES) as tc, ExitStack() as es:
        W = es.enter_context(tc.tile_pool(name="weights", bufs=1))
        S = es.enter_context(tc.tile_pool(name="work", bufs=2))
        P = es.enter_context(tc.tile_pool(name="persist", bufs=1))
        PS = es.enter_context(tc.tile_pool(name="ps", bufs=1, space="PSUM"))

        # ---- load weights ----
        wc1 = W.tile([100, 128], BF16)
        nc.sync.dma_start(out=wc1[:], in_=wc1t)
        w2s = W.tile([128, 1152], BF16)
        nc.sync.dma_start(out=w2s[:], in_=w2t)
        w3 = W.tile([128, 4096], BF16)
        nc.sync.dma_start(out=w3[:], in_=w3t)
        fc2 = W.tile([128, 40], F32)
        nc.sync.dma_start(out=fc2[:], in_=fc2t)
        b1r = W.tile([128, 1], F32)
        nc.sync.dma_start(out=b1r[:], in_=b1t)
        b2 = W.tile([64, 1], F32)
        nc.sync.dma_start(out=b2[:], in_=b2t)
        b3 = W.tile([128, 4], F32)
        nc.sync.dma_start(out=b3[:], in_=b3t)
        fc2b = W.tile([1, 10], F32)
        nc.sync.dma_start(out=fc2b[:], in_=fbt)
        ones1 = W.tile([1, 128], F32)
        nc.vector.memset(ones1[:], 1.0)

        h2 = P.tile([64, 16 * BPC], BF16)          # free = (sp outer, b inner)

        def load_x(blk):
            # conv1 input: 25 shifted replicas per chunk straight from DRAM
            # partition 25j+5kh+kw = chunk j shifted by 28*kh + kw.
            xrep = S.tile([100, XP], BF16, tag="xrep", bufs=3, name="xrep")
            for j in range(4):
                srcR = bass.AP(
                    tensor=xt.tensor,
                    offset=blk * XBLK + j * CHUNKF,
                    ap=[[28, 5], [1, 5], [1, CHUNKF]],
                )
                nc.sync.dma_start(
                    out=xrep[25 * j : 25 * j + 25, 0:CHUNKF], in_=srcR
                )
            return xrep

        def conv1_block(blk, xrep, prev):
            # psum partition m = 32*j + ci (j = chunk, ci = conv1 out ch)
            h1p = S.tile([128, P1], BF16, tag="h1p", name="h1p")
            # zero the pad tail once per block: the shifted copies read
            # up to 13 elements past H1F and NaN*0 = NaN in the PE.
            nc.vector.memset(h1p[:, H1F:P1], 0.0)
            m0 = 0
            c1t = None
            pstart = 0
            for g, gs in enumerate(GSIZES):
                ps1 = PS.tile([128, 1536], F32, tag="ps1", bufs=2, name="ps1")
                for i in range(gs):
                    m = m0 + i
                    s, h = m // 2, m % 2
                    rhs = bass.AP(
                        tensor=xrep[:].tensor,
                        offset=xrep[:].offset + s * 784 + h * 336,
                        ap=[[XP, 100], [28, 12], [1, 24]],
                    )
                    nc.tensor.matmul(
                        out=ps1[:, 512 * i : 512 * i + 288],
                        lhsT=wc1[:],
                        rhs=rhs,
                        start=True,
                        stop=True,
                    )
                # batched bias+relu+cast eviction over the group's banks;
                # c1t holds two groups (6 matmuls) so the pool reduce
                # amortizes its fixed overhead over 1728 elements.
                if c1t is None:
                    c1t = S.tile([128, 1728], BF16, tag="c1t", bufs=2,
                                 name="c1t")
                    pstart = m0
                cbase = (m0 - pstart) * 288
                pin = bass.AP(
                    tensor=ps1[:].tensor,
                    offset=ps1[:].offset,
                    ap=[[1536, 128], [512, gs], [1, 288]],
                )
                cout = bass.AP(
                    tensor=c1t[:].tensor,
                    offset=c1t[:].offset + cbase,
                    ap=[[1728, 128], [288, gs], [1, 288]],
                )
                if ENG_EVICT[g] == 'A':
                    nc.scalar.activation(cout, pin, RELU, bias=b1r[:])
                else:
                    nc.vector.tensor_scalar(
                        out=cout, in0=pin, scalar1=b1r[:], scalar2=0.0,
                        op0=ADDOP, op1=MAXOP,
                    )
                # 2x2 max pool once per c1t fill (6 mms, or the 2mm tail):
                # c1t free = (m_local, ph, s1, pw, s0)
                #   f = ml*288 + ph*48 + s1*24 + pw*2 + s0
                # (ml, ph) merge: stride 48, size 6*nm.
                nm = m0 + gs - pstart
                if nm == 6 or g == len(GSIZES) - 1:
                    ov = bass.AP(
                        tensor=h1p[:].tensor,
                        offset=h1p[:].offset + pstart * 72,
                        ap=[[P1, 128], [12, 6 * nm], [1, 12]],
                    )
                    pv = bass.AP(
                        tensor=c1t[:].tensor,
                        offset=c1t[:].offset,
                        ap=[[1728, 128], [48, 6 * nm], [2, 12], [24, 2], [1, 2]],
                    )
                    nc.vector.tensor_reduce(out=ov, in_=pv, axis=AXY, op=MAXOP)
                    c1t = None
                # interleave previous block's conv2 chunks into the PE stream
                if prev is not None and g in (1, 3, 5, 7):
                    conv2_j(prev[0], prev[1], (1, 3, 5, 7).index(g))
                m0 += gs

            # 4 shifted SBUF->SBUF copies: h1r partition 32c+ci holds
            # chunk j at free j*H1F, shifted by 12*alpha + beta.
            h1r = S.tile([128, PR], BF16, tag="h1r", bufs=3, name="h1r")
            for c, (al, be) in enumerate(COPIES):
                sc = 12 * al + be
                for j in range(4):
                    # partition dims must be a clean outermost run: one
                    # DMA per (copy, chunk), 32 partitions each.
                    src = bass.AP(
                        tensor=h1p[:].tensor,
                        offset=h1p[:].offset + 32 * j * P1 + sc,
                        ap=[[P1, 32], [1, H1F]],
                    )
                    dst = bass.AP(
                        tensor=h1r[:].tensor,
                        offset=h1r[:].offset + 32 * c * PR + j * H1F,
                        ap=[[PR, 32], [1, H1F]],
                    )
                    nc.sync.dma_start(out=dst, in_=src)
            if DEBUG and blk == 0:
                nc.gpsimd.dma_start(out=dbg1, in_=h1p[:])
                nc.gpsimd.dma_start(out=dbg2, in_=h1r[:])
            return h1r

        def conv2_j(blk, h1r, j):
            # 9 delta-packed passes, K=128 each, one N=512 matmul per pass
            ps2 = PS.tile([128, 512], F32, tag="ps2", bufs=2, name="ps2")
            for p, (a, b) in enumerate(PASSES):
                rhs = bass.AP(
                    tensor=h1r[:].tensor,
                    offset=h1r[:].offset + j * H1F + 12 * a + b,
                    ap=[[PR, 128], [144, 16], [24, 4], [1, 8]],
                )
                nc.tensor.matmul(
                    out=ps2[:],
                    lhsT=w2s[:, 128 * p : 128 * p + 128],
                    rhs=rhs,
                    start=(p == 0),
                    stop=(p == 8),
                )
            if DEBUG and blk == 0 and j in (1, 3):
                d4t = S.tile([128, 512], F32, tag="d4t", name="d4t")
                nc.scalar.activation(
                    d4t[:], ps2[:],
                    mybir.ActivationFunctionType.Identity,
                )
                nc.gpsimd.dma_start(out=(dbg4 if j == 1 else dbg5), in_=d4t[:])
            # pool rows: max over delta halves (partitions p vs p+64),
            # bias folded in; one PSUM input per instruction
            t2 = S.tile([64, 512], F32, tag="t2", name="t2")
            nc.scalar.activation(
                t2[:], ps2[64:128, :],
                mybir.ActivationFunctionType.Identity, bias=b2[:],
            )
            m1 = S.tile([64, 512], BF16, tag="m1", name="m1")
            nc.vector.scalar_tensor_tensor(
                out=m1[:], in0=ps2[0:64, :], scalar=b2[:], in1=t2[:],
                op0=ADDOP, op1=MAXOP,
            )
            # pool cols: max(m1[2oc'], 0, m1[2oc'+1]) -> relu folded,
            # written straight into h2[co, sp*BPC + b]
            b0 = blk * BLK + j * CS
            ia = bass.AP(
                tensor=m1[:].tensor,
                offset=m1[:].offset,
                ap=[[512, 64], [8, 4], [2, 4], [32, 16]],
            )
            ib = bass.AP(
                tensor=m1[:].tensor,
                offset=m1[:].offset + 1,
                ap=[[512, 64], [8, 4], [2, 4], [32, 16]],
            )
            outv = bass.AP(
                tensor=h2[:].tensor,
                offset=h2[:].offset + b0,
                ap=[[16 * BPC, 64], [4 * BPC, 4], [BPC, 4], [1, 16]],
            )
            nc.vector.scalar_tensor_tensor(
                out=outv, in0=ia, scalar=0.0, in1=ib, op0=MAXOP, op1=MAXOP
            )

        hr = [P.tile([128, BPC + 8], BF16, tag=f"hr{k}", name=f"hr{k}")
              for k in range(8)]

        def relayout_half(bh):
            for k in range(8):
                src = bass.AP(
                    tensor=h2[:].tensor,
                    offset=h2[:].offset + 8 * k * 16 * BPC + bh * (BPC // 2),
                    ap=[[16 * BPC, 8], [BPC, 16], [1, BPC // 2]],
                )
                nc.sync.dma_start(
                    out=hr[k][:, bh * (BPC // 2) : (bh + 1) * (BPC // 2)],
                    in_=src,
                )

        # ---- main pipeline ----
        xreps = [load_x(0), load_x(1)]
        h1rs = []
        for blk in range(NBLK):
            if blk + 2 < NBLK:
                xreps.append(load_x(blk + 2))
            prev = (blk - 2, h1rs[blk - 2]) if blk >= 2 else None
            h1rs.append(conv1_block(blk, xreps[blk], prev))
        for j in range(4):
            conv2_j(NBLK - 2, h1rs[NBLK - 2], j)
        relayout_half(0)
        for j in range(4):
            conv2_j(NBLK - 1, h1rs[NBLK - 1], j)
        relayout_half(1)
        if DEBUG:
            nc.gpsimd.dma_start(out=dbg3, in_=h2[:])

        # ---- fc1: 8 K-chunks, batch halves so half 0 overlaps tail ----
        h3 = [P.tile([128, BPC], F32, tag=f"h3{m}", name=f"h3{m}")
              for m in range(4)]
        HB = BPC // 2
        for bh in range(2):
            for mh in range(2):
                psf = [PS.tile([128, 512], F32, tag="ps2", bufs=2,
                               name=f"psf{bh}{mh}{m}") for m in range(2)]
                for k in range(8):
                    for mm in range(2):
                        m = 2 * mh + mm
                        nc.tensor.matmul(
                            out=psf[mm][:, 0:HB],
                            lhsT=w3[:, (k * 4 + m) * 128 : (k * 4 + m) * 128 + 128],
                            rhs=hr[k][:, bh * HB : (bh + 1) * HB],
                            start=(k == 0),
                            stop=(k == 7),
                        )
                for mm in range(2):
                    m = 2 * mh + mm
                    nc.scalar.activation(
                        h3[m][:, bh * HB : (bh + 1) * HB],
                        psf[mm][:, 0:HB], RELU, bias=b3[:, m : m + 1],
                    )

        # ---- fc2 + log_softmax, batch on partitions ----
        for bc in range(4):
            psl = PS.tile([128, 10], F32, tag="ps1", bufs=2)
            for k in range(4):
                nc.tensor.matmul(
                    out=psl[:],
                    lhsT=h3[k][:, bc * 128 : bc * 128 + 128],
                    rhs=fc2[:, k * 10 : k * 10 + 10],
                    start=(k == 0),
                    stop=False,
                )
            nc.tensor.matmul(
                out=psl[:],
                lhsT=ones1[:],
                rhs=fc2b[:],
                start=False,
                stop=True,
            )
            negm = S.tile([128, 1], F32, tag="negm")
            nc.vector.tensor_reduce(
                out=negm[:], in_=psl[:], axis=AX, op=MAXOP, negate=True
            )
            shifted = S.tile([128, 10], F32, tag="shifted")
            nc.vector.tensor_scalar(
                out=shifted[:], in0=psl[:], scalar1=negm[:], scalar2=None, op0=ADDOP
            )
            ex = S.tile([128, 10], F32, tag="ex")
            se = S.tile([128, 1], F32, tag="se")
            nc.scalar.activation(ex[:], shifted[:], EXP, accum_out=se[:])
            lse = S.tile([128, 1], F32, tag="lse")
            nc.scalar.activation(lse[:], se[:], LN)
            osb = S.tile([128, 10], F32, tag="osb")
            nc.vector.tensor_scalar(
                out=osb[:], in0=shifted[:], scalar1=lse[:], scalar2=None, op0=SUBOP
            )
            nc.sync.dma_start(out=ot[bc * 128 : bc * 128 + 128, :], in_=osb[:])

    nc.finalize()
    return nc


def _prep_weights(inputs):
    """Host-side: densify sketch weights and lay them out for the kernel."""
    h1, h2i, h3i = inputs["hash_idx1"], inputs["hash_idx2"], inputs["hash_idx3"]
    s1, s2, s3 = inputs["sgn1"], inputs["sgn2"], inputs["sgn3"]
    w1, w2, w3 = inputs["w1"], inputs["w2"], inputs["w3"]
    b1, b2, b3 = inputs["b1"], inputs["b2"], inputs["b3"]
    fc2w, fc2b = inputs["fc2_w"], inputs["fc2_b"]

    wc1 = (w1[:, h1] * s1[None, :]).astype(np.float32)            # (32, 25)
    wc2 = (w2[:, h2i] * s2[None, :]).astype(np.float32).reshape(64, 32, 5, 5)
    W3 = (w3[:, h3i] * s3[None, :]).astype(np.float32)            # (512, 1024)

    # conv1 block-diagonal; psum partition m = 32*j + ci
    wc1bd = np.zeros((100, 128), np.float32)
    for j in range(4):
        for ci in range(32):
            wc1bd[25 * j : 25 * j + 25, 32 * j + ci] = wc1[ci]
    b1r = np.tile(np.asarray(b1, np.float32), 4).reshape(128, 1)

    # conv2 2x2-packed weights: 9 passes (a,b); K row = 32*(2*al+be)+ci,
    # M col = 64*delta+co; tap kh = a+al-delta, kw = b+be.
    w2s = np.zeros((128, 9, 128), np.float32)
    for p, (a, b) in enumerate(PASSES):
        for c, (al, be) in enumerate(COPIES):
            for d in range(2):
                kh = a + al - d
                kw = b + be
                if 0 <= kh <= 4 and 0 <= kw <= 4:
                    w2s[32 * c : 32 * c + 32, p, 64 * d : 64 * d + 64] = \
                        wc2[:, :, kh, kw].T
    w2s = w2s.reshape(128, 1152)

    # fc1: lhsT chunk (k,m) = W3.T[128k:128k+128, 128m:128m+128]
    w3sb = np.zeros((128, 8, 4, 128), np.float32)
    W3T = np.ascontiguousarray(W3.T)  # (1024, 512)
    for k in range(8):
        for m in range(4):
            w3sb[:, k, m, :] = W3T[128 * k : 128 * k + 128, 128 * m : 128 * m + 128]
    w3sb = w3sb.reshape(128, 4096)

    fc2sb = np.zeros((128, 4, 10), np.float32)
    for k in range(4):
        fc2sb[:, k, :] = fc2w[:, 128 * k : 128 * k + 128].T
    fc2sb = fc2sb.reshape(128, 40)

    b3sb = np.asarray(b3, np.float32).reshape(4, 128).T.copy()

    bf = lambda a: np.asarray(a, dtype=ml_dtypes.bfloat16)
    f = lambda a: np.ascontiguousarray(a, dtype=np.float32)
    return {
        "wc1bd": bf(wc1bd),
        "w2s": bf(w2s),
        "w3sb": bf(w3sb),
        "fc2sb": f(fc2sb),
        "b1r": f(b1r),
        "b2": f(np.asarray(b2).reshape(64, 1)),
        "b3sb": f(b3sb),
        "fc2b": f(np.asarray(fc2b).reshape(1, 10)),
    }


def kernel(**inputs):
    out, _ = _run(inputs, trace=False)
    return out


def _run(inputs, trace=False):
    if "nc" not in _CACHE:
        _CACHE["nc"] = _build()
    nc = _CACHE["nc"]

    wmap = _prep_weights(inputs)
    x = np.asarray(inputs["x"], np.float32).reshape(4096, 784)

    in_maps = []
    for c in range(NCORES):
        xs = x[c * BPC : (c + 1) * BPC].reshape(-1)
        xs = np.concatenate([xs, np.zeros(XPAD, np.float32)])
        m = dict(wmap)
        m["x"] = np.asarray(xs, dtype=ml_dtypes.bfloat16)
        in_maps.append(m)

    res = run_bass_kernel_spmd(
        nc, in_maps, core_ids=list(range(NCORES)), trace=trace
    )
    out = np.concatenate([res.results[c]["out"] for c in range(NCORES)], axis=0)
    return out.astype(np.float32), res
